# revision 75
# baseline (speedup 1.0000x reference)
"""Trainium2 Bass kernel for nn_ActorNetwork (gnn_message_passing).

Pure data-parallel across 8 NeuronCores: each core processes 8192 of the
65536 batch rows; small weights are replicated.

v6: feature-major, neighbor-pair packing; deep software pipeline tuned so
the in-order PE stream never waits on slower engines:
  - state1 uploaded pre-transposed (bf16 [8, 128, rpc]); prefetched two
    macros ahead.
  - state0 + state2 host-packed into one 64-col bf16 block per row
    (56 s2 | 6 s0 | 2 pad); per-neighbor mask sums via a jsum matmul after
    the PE transpose; mask folded into the score PSUM via an identity
    matmul over the -1e30 rows.
  - outputs PE-transposed into a [128, 128] per-core block -> one 512B/part
    store DMA; the out-head matmul + tanh + transpose are pipelined 2-3
    macros behind compute.
  - all small weights packed into two DMAs issued on the Act queue.
  - elementwise work balanced across Act / DVE / Pool.
"""

import os

import numpy as np
import ml_dtypes

import concourse.bass as bass
import concourse.tile as tile
from concourse import bacc
from concourse import mybir
from concourse.bass_utils import run_bass_kernel_spmd

F32 = mybir.dt.float32
F32R = mybir.dt.float32r
BF16 = mybir.dt.bfloat16
F8 = mybir.dt.float8e4
DR = mybir.MatmulPerfMode.DoubleRow

N_CORES = 8
B_FULL = 65536
RPC = B_FULL // N_CORES        # rows per core = 8192
MACRO = 512                    # batch rows per macro tile
P = 128

Relu = mybir.ActivationFunctionType.Relu
Tanh = mybir.ActivationFunctionType.Tanh
Exp = mybir.ActivationFunctionType.Exp
Alu = mybir.AluOpType
AX = mybir.AxisListType

# column layout of the packed bf16 weight block [128, WPK_COLS]
_off = 0
def _span(n):
    global _off
    s = (_off, _off + n)
    _off += n
    return s
WG_S = _span(512)        # [128, (8c, 64)]
WS1_S = _span(512)       # [62, (4m, 128)]
WS2_S = _span(128)       # [128, 128]
WQK_S = _span(128)       # [64, 128]
WC1A_S = _span(128)      # [128, 128]
WVC_S = _span(128)       # [128, 128]
WC2_S = _span(128)       # [128, 128]
WC3_S = _span(2)         # [128, 2]
SEL_S = _span(512)       # [8, (4m, 128)]
SSEL_S = _span(32)       # [128, (4m, 8)]
ONES8_S = _span(1)       # [8, 1]
ID8_S = _span(8)         # [8, 8]
JOWN_S = _span(64)       # [62, 64] w0 rows 56:62
IDB_S = _span(128)       # [128, 128] bf16 identity
WPK_COLS = _off


def build(rpc=RPC, macro=MACRO):
    nm = rpc // macro          # macro tiles per core = 16
    nb = macro // P            # 128-row blocks per macro tile = 4

    nc = bacc.Bacc()

    s1t_d = nc.declare_dram_parameter("s1t8", [2, 8, P, rpc], F8, isOutput=False)
    s2a_d = nc.declare_dram_parameter("s2aug", [P, nm, nb * 128], BF16, isOutput=False)
    wpk_d = nc.declare_dram_parameter("wpk", [P, WPK_COLS], BF16, isOutput=False)
    w8pk_d = nc.declare_dram_parameter("w8pk", [P, 1024], F8, isOutput=False)
    bpk_d = nc.declare_dram_parameter("bpk", [P, 10], F32, isOutput=False)
    out_d = nc.declare_dram_parameter("out", [P, nm * nb * 2], F32, isOutput=True)

    from concourse import library_config

    with tile.TileContext(nc) as tc:
        nc.gpsimd.load_library(library_config.proxy)
        consts = tc.alloc_tile_pool(name="consts", bufs=1)
        s1T_p = tc.alloc_tile_pool(name="s1T", bufs=3)
        s2_p = tc.alloc_tile_pool(name="s2", bufs=3)
        sm_p = tc.alloc_tile_pool(name="sm", bufs=4)
        work_p = tc.alloc_tile_pool(name="work", bufs=4)
        blk_p = tc.alloc_tile_pool(name="blk", bufs=4)
        psA2 = tc.alloc_tile_pool(name="psA2", bufs=2, space="PSUM")
        psJ = tc.alloc_tile_pool(name="psJ", bufs=1, space="PSUM")
        psB = tc.alloc_tile_pool(name="psB", bufs=1, space="PSUM")
        psB2 = tc.alloc_tile_pool(name="psB2", bufs=1, space="PSUM")
        psC = tc.alloc_tile_pool(name="psC", bufs=1, space="PSUM")

        # ---- packed constants (Act queue; SP starts on state immediately) --
        wpk = consts.tile([P, WPK_COLS], BF16)
        nc.scalar.dma_start(wpk, wpk_d[:, :])
        w8pk = consts.tile([P, 2, 4, 2, 64], F8)
        nc.scalar.dma_start(
            w8pk.rearrange("p h c t m -> p (h c t m)"), w8pk_d[:, :]
        )
        bpk = consts.tile([P, 10], F32)
        nc.scalar.dma_start(bpk, bpk_d[:, :])

        def W(span, rows=P):
            return wpk[0:rows, span[0] : span[1]]

        wg_sb = W(WG_S).rearrange("p (c m) -> p c m", c=8)
        ws1_sb = W(WS1_S, 62).rearrange("p (m k) -> p m k", m=4)
        ws2_sb = W(WS2_S)
        wqk_sb = W(WQK_S, 64)
        wc1a_sb = W(WC1A_S)
        wvc_sb = W(WVC_S)
        wc2_sb = W(WC2_S)
        wc3_sb = W(WC3_S)
        sel_sb = W(SEL_S, 8).rearrange("p (m k) -> p m k", m=4)
        ssel_sb = W(SSEL_S).rearrange("p (m k) -> p m k", m=4)
        ones8_sb = W(ONES8_S, 8)
        ident8_sb = W(ID8_S, 8)
        jown_sb = W(JOWN_S, 62)
        ident_b = W(IDB_S)

        b0bg_sb = bpk[:, 0:1]
        bs1_sb = bpk[:, 1:2]
        bs2_sb = bpk[:, 2:3]
        bc1_sb = bpk[:, 3:4]
        bc2_sb = bpk[:, 4:5]
        bc3_sb = bpk[0:2, 5:6]
        ident2_f = bpk[0:2, 6:8]

        out128_sb = consts.tile([P, nm, nb, 2], F32)

        def emit_F1(ms):
            """State loads for macro ms (prefetched two macros ahead)."""
            row0 = ms * macro
            s2a = s2_p.tile([P, nb, 128], BF16, tag="s2a")
            nc.sync.dma_start(
                s2a.rearrange("p o k -> p (o k)"), s2a_d[:, ms, :]
            )
            s1T = s1T_p.tile([P, 2, 8, macro], F8, tag="s1T")
            nc.sync.dma_start(
                s1T,
                s1t_d[:, :, :, row0 : row0 + macro].rearrange(
                    "h c p b -> p h c b"
                ),
            )
            return dict(row0=row0, s1T=s1T, s2a=s2a)

        def emit_OUT(st):
            """Output head + tanh (for macro m-2)."""
            o_ps = psB.tile([2, macro], F32, tag="psB")
            nc.tensor.matmul(o_ps, wc3_sb, st["h2_sb"], start=True, stop=True)
            o_sb = sm_p.tile([2, macro], F32, tag="osb")
            nc.scalar.activation(o_sb, o_ps, Tanh, bias=bc3_sb)
            st["o_sb"] = o_sb

        def emit_B1a(st):
            """Scores (incl. mask) -> exp."""
            sc_ps = psC.tile([8, macro], F32, tag="psC")
            for m in range(4):
                nc.tensor.matmul(
                    sc_ps, ssel_sb[:, m, :], st["qk_sb"][:, m, :],
                    start=(m == 0), stop=False,
                )
            nc.tensor.matmul(sc_ps, ident8_sb, st["nm_sb"], start=False, stop=True)
            p8_sb = sm_p.tile([8, macro], BF16, tag="p8")
            nc.scalar.activation(p8_sb, sc_ps, Exp, scale=0.125)
            st["p8_sb"] = p8_sb

        def emit_F1t(st):
            """mask sums (batch-major) + xbar DMA transposes straight to SBUF."""
            s2a = st["s2a"]
            with nc.allow_low_precision(reason="mask sums only compared to 0"):
                nc.vector.tensor_reduce(
                    s2a[:, :, 64:72],
                    s2a[:, :, 0:56].rearrange("p o (n j) -> p o n j", j=7),
                    AX.X,
                    Alu.add,
                )
            s2T_sb = sm_p.tile([P, nb, P], BF16, tag="s2Tsb")
            for o in range(nb):
                nc.sync.dma_start_transpose(s2T_sb[:, o, :], s2a[:, o, :])
            st["s2T_sb"] = s2T_sb

        def emit_F2a(st):
            """env matmuls: fp8 DoubleRow, hi@Wa + hi@Wb + lo@Wa."""
            env_ps = psB.tile([64, macro], F32, tag="psB")
            s1T = st["s1T"]
            first = True
            for h, w in ((0, 0), (0, 1), (1, 0)):
                for c4 in range(4):
                    nc.tensor.matmul(
                        env_ps,
                        w8pk[:, w, c4],
                        s1T[:, h, 2 * c4 : 2 * c4 + 2, :],
                        start=first, stop=(h, w, c4) == (1, 0, 3),
                        perf_mode=DR,
                    )
                    first = False
            st["env_ps"] = env_ps

        def emit_B1b1(st):
            """Softmax denominator."""
            den_ps = psC.tile([1, macro], F32, tag="psC")
            nc.tensor.matmul(den_ps, ones8_sb, st["p8_sb"], start=True, stop=True)
            rs_sb = sm_p.tile([1, macro], F32R, tag="rs")
            with nc.allow_low_precision(reason="f32r reciprocal, 19-bit ok"):
                nc.vector.reciprocal(rs_sb, den_ps)
            st["rs_sb"] = rs_sb

        def emit_F2b(st):
            """own matmul; mask rows; concatA; q2."""
            s2T_sb = st["s2T_sb"]
            s2T_flat = s2T_sb[0:62].rearrange("p o k -> p (o k)")
            st["s2T_flat"] = s2T_flat

            nm_sb = sm_p.tile([8, macro], BF16, tag="nm")
            nc.vector.tensor_scalar(
                nm_sb,
                s2T_sb[64:72].rearrange("p o k -> p (o k)"),
                0.0, -1e30, Alu.is_equal, Alu.mult,
            )
            st["nm_sb"] = nm_sb

            jo_ps = psJ.tile([64, macro], F32, tag="psJ")
            nc.tensor.matmul(jo_ps, jown_sb, s2T_flat, start=True, stop=True)
            concatA = work_p.tile([P, macro], BF16, tag="concatA")
            nc.scalar.activation(
                concatA[0:64, :], jo_ps[0:64], Relu, bias=b0bg_sb[0:64]
            )
            nc.scalar.activation(
                concatA[64:128, :], st["env_ps"], Relu,
                bias=b0bg_sb[64:128], scale=1.0 / 32.0,
            )

            q2_ps = psB.tile([P, macro], F32, tag="psB")
            nc.tensor.matmul(q2_ps, wqk_sb, concatA[0:64, :], start=True, stop=True)
            q2_sb = work_p.tile([P, macro], BF16, tag="q2")
            nc.scalar.copy(q2_sb, q2_ps)
            st["concatA"] = concatA
            st["q2_sb"] = q2_sb

        def emit_B1b2(st):
            """1/den broadcast -> alpha."""
            rdup_sb = sm_p.tile([8, macro], F32R, tag="rdup")
            nc.gpsimd.partition_broadcast(rdup_sb, st["rs_sb"], channels=8)
            alpha_sb = sm_p.tile([8, macro], BF16, tag="alpha")
            nc.gpsimd.tensor_tensor(alpha_sb, st["p8_sb"], rdup_sb, Alu.mult)
            st["alpha_sb"] = alpha_sb

        def emit_F3a(st):
            """i1."""
            i1_sb = blk_p.tile([P, 4, macro], BF16, tag="i1")
            for t in range(2):
                pair_ps = psA2.tile([P, 2, macro], F32, tag="psA2")
                for h in range(2):
                    nc.tensor.matmul(
                        pair_ps[:, h, :], ws1_sb[:, 2 * t + h],
                        st["s2T_flat"], start=True, stop=True,
                    )
                dst = i1_sb[:, 2 * t : 2 * t + 2, :]
                if t == 0:
                    nc.scalar.activation(dst, pair_ps, Relu, bias=bs1_sb)
                else:
                    nc.vector.tensor_scalar(
                        dst, pair_ps, bs1_sb, 0.0, Alu.add, Alu.max
                    )
            st["i1_sb"] = i1_sb

        def emit_B2a(st):
            """Weighted i2 -> h1."""
            i2_sb = st["i2_sb"]; alpha_sb = st["alpha_sb"]

            cmul_sb = blk_p.tile([P, 4, macro], BF16, tag="cmul")
            for t in range(2):
                abc_ps = psA2.tile([P, 2, macro], F32, tag="psA2")
                for h in range(2):
                    nc.tensor.matmul(
                        abc_ps[:, h, :], sel_sb[:, 2 * t + h], alpha_sb,
                        start=True, stop=True,
                    )
                nc.vector.tensor_tensor(
                    cmul_sb[:, 2 * t : 2 * t + 2, :],
                    i2_sb[:, 2 * t : 2 * t + 2, :], abc_ps, Alu.mult,
                )

            h1_ps = psB2.tile([P, macro], F32, tag="psB2")
            nc.tensor.matmul(h1_ps, wc1a_sb, st["concatA"], start=True, stop=False)
            for m in range(4):
                nc.tensor.matmul(
                    h1_ps, wvc_sb, cmul_sb[:, m, :],
                    start=False, stop=(m == 3),
                )
            h1_sb = work_p.tile([P, macro], BF16, tag="h1")
            nc.scalar.activation(h1_sb, h1_ps, Relu, bias=bc1_sb)
            st["h1_sb"] = h1_sb

        def emit_F3b(st):
            """i2, qk."""
            i1_sb = st["i1_sb"]; q2_sb = st["q2_sb"]
            i2_sb = blk_p.tile([P, 4, macro], BF16, tag="i2")
            for t in range(2):
                pair_ps = psA2.tile([P, 2, macro], F32, tag="psA2")
                for h in range(2):
                    nc.tensor.matmul(
                        pair_ps[:, h, :], ws2_sb, i1_sb[:, 2 * t + h, :],
                        start=True, stop=True,
                    )
                dst = i2_sb[:, 2 * t : 2 * t + 2, :]
                if t == 0:
                    nc.scalar.activation(dst, pair_ps, Relu, bias=bs2_sb)
                else:
                    nc.vector.tensor_scalar(
                        dst, pair_ps, bs2_sb, 0.0, Alu.add, Alu.max
                    )

            qk_sb = blk_p.tile([P, 4, macro], BF16, tag="qk")
            nc.gpsimd.tensor_tensor(
                qk_sb[:, 0:2, :], i2_sb[:, 0:2, :],
                q2_sb[:, None, :].to_broadcast((P, 2, macro)),
                Alu.mult,
            )
            nc.gpsimd.tensor_tensor(
                qk_sb[:, 2:4, :], i2_sb[:, 2:4, :],
                q2_sb[:, None, :].to_broadcast((P, 2, macro)),
                Alu.mult,
            )
            st["i2_sb"] = i2_sb
            st["qk_sb"] = qk_sb

        def emit_B3(st):
            """Output transpose into the 128-partition store block."""
            ms = st["row0"] // macro
            oT_ps = psC.tile([P, nb, 2], F32, tag="psC")
            for o in range(nb):
                nc.tensor.transpose(
                    oT_ps[:, o, :], st["o_sb"][:, o * P : (o + 1) * P], ident2_f
                )
            nc.vector.tensor_copy(out128_sb[:, ms], oT_ps)

        def emit_B2b(st):
            """h2."""
            h2_ps = psB2.tile([P, macro], F32, tag="psB2")
            nc.tensor.matmul(h2_ps, wc2_sb, st["h1_sb"], start=True, stop=True)
            h2_sb = work_p.tile([P, macro], BF16, tag="h2")
            nc.scalar.activation(h2_sb, h2_ps, Relu, bias=bc2_sb)
            st["h2_sb"] = h2_sb

        # software pipeline across macros:
        #   iter i: forward phases for macro i, softmax (B1) for i-1,
        #   weighted-sum/head (B2) for i-2, output head for i-3, output
        #   transpose for i-4. Each chain segment gets a full iteration of
        #   slack. DMAs prefetch 2 ahead.
        sts = [None] * (nm + 1)
        sts[0] = emit_F1(0)
        sts[1] = emit_F1(1)

        def stage(ms):
            return sts[ms] if 0 <= ms < nm else None

        for ms in range(nm + 5):
            cur = stage(ms)
            p1, p2, p3, p4 = (
                stage(ms - 1), stage(ms - 2), stage(ms - 3), stage(ms - 4)
            )
            if p1 is not None:
                emit_B1a(p1)
            if cur is not None:
                emit_F1t(cur)
            if ms + 2 < nm:
                sts[ms + 2] = emit_F1(ms + 2)
            if cur is not None:
                emit_F2a(cur)
            if p1 is not None:
                emit_B1b1(p1)
            if cur is not None:
                emit_F2b(cur)
            if p3 is not None:
                emit_OUT(p3)
            if p1 is not None:
                emit_B1b2(p1)
            if cur is not None:
                emit_F3a(cur)
            if p2 is not None:
                emit_B2a(p2)
            if cur is not None:
                emit_F3b(cur)
            if p4 is not None:
                emit_B3(p4)
            if p2 is not None:
                emit_B2b(p2)

        nc.sync.dma_start(
            out_d[:, :], out128_sb.rearrange("p a o k -> p (a o k)")
        )

        for _pool in (psC, psB2, psB, psJ, psA2, blk_p, work_p, sm_p,
                      s2_p, s1T_p, consts):
            _pool.release()

    return nc


def prepare_in_maps(inputs):
    bf = ml_dtypes.bfloat16
    f32 = np.float32
    nm = RPC // MACRO
    nb = MACRO // P

    def a(x, dt=f32):
        return np.ascontiguousarray(np.asarray(x), dtype=dt)

    W0 = a(inputs["W0"]); Wg = a(inputs["Wg"])
    Ws1 = a(inputs["Ws1"]); Ws2 = a(inputs["Ws2"])
    Wq = a(inputs["Wq"]); Wk = a(inputs["Wk"]); Wv = a(inputs["Wv"])
    Wc1 = a(inputs["Wc1"]); Wc2 = a(inputs["Wc2"]); Wc3 = a(inputs["Wc3"])

    f8 = ml_dtypes.float8_e4m3fn
    wqk = Wq @ Wk.T                                   # [64, 64]
    wvc = Wv @ Wc1[128:192, :]                        # [64, 128]

    wpk = np.zeros((P, WPK_COLS), dtype=f32)

    def put(span, arr):
        arr = np.asarray(arr, dtype=f32)
        wpk[0 : arr.shape[0], span[0] : span[0] + arr.shape[1]] = arr

    # wg fp8 hi/lo split, scaled by 32 into e4m3's normal range; layout
    # [p, (hi/lo, c4, t, 64)] with chunk index c = 2*c4 + t
    wg32 = Wg * 32.0
    wg_hi = wg32.astype(f8)
    wg_lo = (wg32 - wg_hi.astype(f32)).astype(f8)
    w8pk = np.stack(
        [
            w.reshape(4, 2, P, 64).transpose(2, 0, 1, 3).reshape(P, 512)
            for w in (wg_hi, wg_lo)
        ],
        axis=1,
    ).reshape(P, 1024)
    # ws1_blk[7n+j, m, 64r+d] = Ws1[j, d] if n == 2m+r (rows 56:62 zero)
    ws1_blk = np.zeros((62, 4, 128), dtype=f32)
    sel = np.zeros((8, 4, 128), dtype=f32)
    for n in range(8):
        m, r = n // 2, n % 2
        ws1_blk[7 * n : 7 * n + 7, m, 64 * r : 64 * r + 64] = Ws1
        sel[n, m, 64 * r : 64 * r + 64] = 1.0
    put(WS1_S, ws1_blk.reshape(62, 512))
    jown = np.zeros((62, 64), dtype=f32)
    jown[56:62, :] = W0
    put(JOWN_S, jown)
    ws2_blk = np.zeros((128, 128), dtype=f32)
    ws2_blk[0:64, 0:64] = Ws2
    ws2_blk[64:128, 64:128] = Ws2
    put(WS2_S, ws2_blk)
    put(WQK_S, np.concatenate([wqk, wqk], axis=1))
    put(WC1A_S, Wc1[0:128, :])
    put(WVC_S, np.concatenate([wvc, wvc], axis=0))
    put(WC2_S, Wc2)
    put(WC3_S, Wc3)
    put(SEL_S, sel.reshape(8, 512))
    scoresel = np.zeros((128, 4, 8), dtype=f32)
    for n in range(8):
        m, r = n // 2, n % 2
        scoresel[64 * r : 64 * r + 64, m, n] = 1.0
    put(SSEL_S, scoresel.reshape(128, 32))
    put(ONES8_S, np.ones((8, 1), dtype=f32))
    put(ID8_S, np.eye(8, dtype=f32))
    put(IDB_S, np.eye(128, dtype=f32))

    def col(x):
        return np.ascontiguousarray(np.asarray(x, dtype=f32).reshape(-1, 1))

    bpk = np.zeros((P, 10), dtype=f32)
    bpk[:, 0:1] = np.concatenate([col(inputs["b0"]), col(inputs["bg"])], axis=0)
    bpk[:, 1:2] = np.concatenate([col(inputs["bs1"])] * 2, axis=0)
    bpk[:, 2:3] = np.concatenate([col(inputs["bs2"])] * 2, axis=0)
    bpk[:, 3:4] = col(inputs["bc1"])
    bpk[:, 4:5] = col(inputs["bc2"])
    bpk[0:2, 5:6] = col(inputs["bc3"])
    bpk[0:2, 6:8] = np.eye(2, dtype=f32)
    bpk[0:64, 8] = 1.0          # own rows: unscaled
    bpk[64:128, 8] = 1.0 / 32.0  # env rows: undo the fp8 weight scaling

    state0 = a(inputs["state0"]); state1 = a(inputs["state1"])
    state2 = a(inputs["state2"])

    # state1 pre-transposed + fp8 hi/lo split: s1t8[h, c, p, b]
    s1t_f = state1.T.reshape(8, P, B_FULL)
    s1_hi = s1t_f.astype(f8)
    s1_lo = (s1t_f - s1_hi.astype(f32)).astype(f8)
    s1t_full = np.ascontiguousarray(np.stack([s1_hi, s1_lo], axis=0))

    # s2aug: per row 128 cols = 56 s2 | 6 s0 | 2 pad | 8 mask slots | pad
    s2aug = np.zeros((B_FULL, 128), dtype=bf)
    s2aug[:, 0:56] = state2.reshape(B_FULL, 56).astype(bf)
    s2aug[:, 56:62] = state0.astype(bf)
    # [core, ms, o, p, 128] -> [core, p, ms, (o 128)]
    s2aug_blk = np.ascontiguousarray(
        s2aug.reshape(N_CORES, nm, nb, P, 128).transpose(0, 3, 1, 2, 4)
        .reshape(N_CORES, P, nm, nb * 128)
    )

    shared = {
        "wpk": a(wpk, bf),
        "w8pk": np.ascontiguousarray(w8pk),
        "bpk": bpk,
    }
    in_maps = []
    for i in range(N_CORES):
        m = dict(shared)
        m["s1t8"] = np.ascontiguousarray(
            s1t_full[:, :, :, i * RPC : (i + 1) * RPC]
        )
        m["s2aug"] = s2aug_blk[i]
        in_maps.append(m)
    return in_maps


def unshard_out(res_core):
    """[128, nm*nb*2] f32 -> [rpc, 2]; out128[p, ms, o, a] = row ms*512+o*128+p."""
    nm = RPC // MACRO
    nb = MACRO // P
    arr = np.asarray(res_core, dtype=np.float32).reshape(P, nm, nb, 2)
    return np.ascontiguousarray(
        arr.transpose(1, 2, 0, 3).reshape(RPC, 2)
    )


_NC_CACHE = {}


def get_nc():
    if "nc" not in _NC_CACHE:
        nc = build()
        nc.finalize()
        _NC_CACHE["nc"] = nc
    return _NC_CACHE["nc"]


def kernel(**inputs):
    nc = get_nc()
    in_maps = prepare_in_maps(inputs)
    trace = bool(int(os.environ.get("K_TRACE", "0")))
    try:
        res = run_bass_kernel_spmd(
            nc, in_maps, core_ids=list(range(N_CORES)), trace=trace
        )
    except ModuleNotFoundError:
        res = run_bass_kernel_spmd(nc, in_maps, core_ids=list(range(N_CORES)))
    if res.exec_time_ns is not None:
        print(f"HW exec time: {res.exec_time_ns} ns")
    parts = [unshard_out(res.results[i]["out"]) for i in range(N_CORES)]
    return np.ascontiguousarray(np.concatenate(parts, axis=0))


# revision 85
# speedup vs baseline: 1.0478x; 1.0478x over previous
"""Trainium2 Bass kernel for nn_ActorNetwork (gnn_message_passing).

Pure data-parallel across 8 NeuronCores: each core processes 8192 of the
65536 batch rows; small weights are replicated.

v6: feature-major, neighbor-pair packing; deep software pipeline tuned so
the in-order PE stream never waits on slower engines:
  - state1 uploaded pre-transposed (bf16 [8, 128, rpc]); prefetched two
    macros ahead.
  - state0 + state2 host-packed into one 64-col bf16 block per row
    (56 s2 | 6 s0 | 2 pad); per-neighbor mask sums via a jsum matmul after
    the PE transpose; mask folded into the score PSUM via an identity
    matmul over the -1e30 rows.
  - outputs PE-transposed into a [128, 128] per-core block -> one 512B/part
    store DMA; the out-head matmul + tanh + transpose are pipelined 2-3
    macros behind compute.
  - all small weights packed into two DMAs issued on the Act queue.
  - elementwise work balanced across Act / DVE / Pool.
"""

import os

import numpy as np
import ml_dtypes

import concourse.bass as bass
import concourse.tile as tile
from concourse import bacc
from concourse import mybir
from concourse.bass_utils import run_bass_kernel_spmd

F32 = mybir.dt.float32
F32R = mybir.dt.float32r
BF16 = mybir.dt.bfloat16
F8 = mybir.dt.float8e4
DR = mybir.MatmulPerfMode.DoubleRow

N_CORES = 8
B_FULL = 65536
RPC = B_FULL // N_CORES        # rows per core = 8192
MACRO = 512                    # batch rows per macro tile
P = 128

Relu = mybir.ActivationFunctionType.Relu
Tanh = mybir.ActivationFunctionType.Tanh
Exp = mybir.ActivationFunctionType.Exp
Alu = mybir.AluOpType
AX = mybir.AxisListType

# column layout of the packed bf16 weight block [128, WPK_COLS]
_off = 0
def _span(n):
    global _off
    s = (_off, _off + n)
    _off += n
    return s
WG_S = _span(512)        # [128, (8c, 64)]
WS1_S = _span(512)       # [62, (4m, 128)]
WS2_S = _span(128)       # [128, 128]
WQK_S = _span(128)       # [64, 128]
WC1A_S = _span(128)      # [128, 128]
WVC_S = _span(128)       # [128, 128]
WC2_S = _span(128)       # [128, 128]
WC3_S = _span(2)         # [128, 2]
SEL_S = _span(512)       # [8, (4m, 128)]
SSEL_S = _span(32)       # [128, (4m, 8)]
ONES8_S = _span(1)       # [8, 1]
ID8_S = _span(8)         # [8, 8]
JOWN_S = _span(64)       # [62, 64] w0 rows 56:62
IDB_S = _span(128)       # [128, 128] bf16 identity
WPK_COLS = _off


def build(rpc=RPC, macro=MACRO):
    nm = rpc // macro          # macro tiles per core = 16
    nb = macro // P            # 128-row blocks per macro tile = 4

    nc = bacc.Bacc()

    s1t_d = nc.declare_dram_parameter("s1t8", [2, 8, P, rpc], F8, isOutput=False)
    s2a_d = nc.declare_dram_parameter("s2aug", [P, nm, nb * 128], BF16, isOutput=False)
    wpk_d = nc.declare_dram_parameter("wpk", [P, WPK_COLS], BF16, isOutput=False)
    w8pk_d = nc.declare_dram_parameter("w8pk", [P, 1024], F8, isOutput=False)
    bpk_d = nc.declare_dram_parameter("bpk", [P, 10], F32, isOutput=False)
    out_d = nc.declare_dram_parameter("out", [P, nm * nb * 2], F32, isOutput=True)

    from concourse import library_config

    with tile.TileContext(nc) as tc:
        nc.gpsimd.load_library(library_config.proxy)
        consts = tc.alloc_tile_pool(name="consts", bufs=1)
        s1T_p = tc.alloc_tile_pool(name="s1T", bufs=3)
        s2_p = tc.alloc_tile_pool(name="s2", bufs=3)
        sm_p = tc.alloc_tile_pool(name="sm", bufs=6)
        work_p = tc.alloc_tile_pool(name="work", bufs=6)
        blk_p = tc.alloc_tile_pool(name="blk", bufs=5)
        psA2 = tc.alloc_tile_pool(name="psA2", bufs=2, space="PSUM")
        psJ = tc.alloc_tile_pool(name="psJ", bufs=1, space="PSUM")
        psB = tc.alloc_tile_pool(name="psB", bufs=1, space="PSUM")
        psB2 = tc.alloc_tile_pool(name="psB2", bufs=1, space="PSUM")
        psC = tc.alloc_tile_pool(name="psC", bufs=1, space="PSUM")

        # ---- packed constants (Act queue; SP starts on state immediately) --
        wpk = consts.tile([P, WPK_COLS], BF16)
        nc.scalar.dma_start(wpk, wpk_d[:, :])
        w8pk = consts.tile([P, 2, 4, 2, 64], F8)
        nc.scalar.dma_start(
            w8pk.rearrange("p h c t m -> p (h c t m)"), w8pk_d[:, :]
        )
        bpk = consts.tile([P, 10], F32)
        nc.scalar.dma_start(bpk, bpk_d[:, :])

        def W(span, rows=P):
            return wpk[0:rows, span[0] : span[1]]

        wg_sb = W(WG_S).rearrange("p (c m) -> p c m", c=8)
        ws1_sb = W(WS1_S, 62).rearrange("p (m k) -> p m k", m=4)
        ws2_sb = W(WS2_S)
        wqk_sb = W(WQK_S, 64)
        wc1a_sb = W(WC1A_S)
        wvc_sb = W(WVC_S)
        wc2_sb = W(WC2_S)
        wc3_sb = W(WC3_S)
        sel_sb = W(SEL_S, 8).rearrange("p (m k) -> p m k", m=4)
        ssel_sb = W(SSEL_S).rearrange("p (m k) -> p m k", m=4)
        ones8_sb = W(ONES8_S, 8)
        ident8_sb = W(ID8_S, 8)
        jown_sb = W(JOWN_S, 62)
        ident_b = W(IDB_S)

        b0bg_sb = bpk[:, 0:1]
        bs1_sb = bpk[:, 1:2]
        bs2_sb = bpk[:, 2:3]
        bc1_sb = bpk[:, 3:4]
        bc2_sb = bpk[:, 4:5]
        bc3_sb = bpk[0:2, 5:6]
        ident2_f = bpk[0:2, 6:8]

        out128_sb = consts.tile([P, nm, nb, 2], F32)

        def emit_F1(ms):
            """State loads for macro ms (prefetched two macros ahead)."""
            row0 = ms * macro
            s2a = s2_p.tile([P, nb, 128], BF16, tag="s2a")
            nc.sync.dma_start(
                s2a.rearrange("p o k -> p (o k)"), s2a_d[:, ms, :]
            )
            s1T = s1T_p.tile([P, 2, 8, macro], F8, tag="s1T")
            nc.sync.dma_start(
                s1T,
                s1t_d[:, :, :, row0 : row0 + macro].rearrange(
                    "h c p b -> p h c b"
                ),
            )
            return dict(row0=row0, s1T=s1T, s2a=s2a)

        def emit_OUT(st):
            """Output head + tanh (for macro m-2)."""
            o_ps = psB.tile([2, macro], F32, tag="psB")
            nc.tensor.matmul(o_ps, wc3_sb, st["h2_sb"], start=True, stop=True)
            o_sb = sm_p.tile([2, macro], F32, tag="osb")
            nc.scalar.activation(o_sb, o_ps, Tanh, bias=bc3_sb)
            st["o_sb"] = o_sb

        def emit_B1a(st):
            """Scores (incl. mask) -> exp."""
            sc_ps = psC.tile([8, macro], F32, tag="psC")
            for m in range(4):
                nc.tensor.matmul(
                    sc_ps, ssel_sb[:, m, :], st["qk_sb"][:, m, :],
                    start=(m == 0), stop=False,
                )
            nc.tensor.matmul(sc_ps, ident8_sb, st["nm_sb"], start=False, stop=True)
            p8_sb = sm_p.tile([8, macro], BF16, tag="p8")
            nc.scalar.activation(p8_sb, sc_ps, Exp, scale=0.125)
            st["p8_sb"] = p8_sb

        def emit_F1t(st):
            """mask sums (batch-major) + xbar DMA transposes straight to SBUF."""
            s2a = st["s2a"]
            with nc.allow_low_precision(reason="mask sums only compared to 0"):
                nc.vector.tensor_reduce(
                    s2a[:, :, 64:72],
                    s2a[:, :, 0:56].rearrange("p o (n j) -> p o n j", j=7),
                    AX.X,
                    Alu.add,
                )
            s2T_sb = sm_p.tile([P, nb, P], BF16, tag="s2Tsb")
            for o in range(nb):
                nc.sync.dma_start_transpose(s2T_sb[:, o, :], s2a[:, o, :])
            st["s2T_sb"] = s2T_sb

        def emit_F2a(st):
            """env matmuls: fp8 DoubleRow, hi@Wa + hi@Wb + lo@Wa."""
            env_ps = psB.tile([64, macro], F32, tag="psB")
            s1T = st["s1T"]
            first = True
            for h, w in ((0, 0), (0, 1), (1, 0)):
                for c4 in range(4):
                    nc.tensor.matmul(
                        env_ps,
                        w8pk[:, w, c4],
                        s1T[:, h, 2 * c4 : 2 * c4 + 2, :],
                        start=first, stop=(h, w, c4) == (1, 0, 3),
                        perf_mode=DR,
                    )
                    first = False
            st["env_ps"] = env_ps

        def emit_B1b1(st):
            """Softmax denominator."""
            den_ps = psC.tile([1, macro], F32, tag="psC")
            nc.tensor.matmul(den_ps, ones8_sb, st["p8_sb"], start=True, stop=True)
            rs_sb = sm_p.tile([1, macro], F32R, tag="rs")
            with nc.allow_low_precision(reason="f32r reciprocal, 19-bit ok"):
                nc.vector.reciprocal(rs_sb, den_ps)
            st["rs_sb"] = rs_sb

        def emit_F2b(st):
            """own matmul; mask rows; concatA; q2."""
            s2T_sb = st["s2T_sb"]
            s2T_flat = s2T_sb[0:62].rearrange("p o k -> p (o k)")
            st["s2T_flat"] = s2T_flat

            nm_sb = sm_p.tile([8, macro], BF16, tag="nm")
            nc.vector.tensor_scalar(
                nm_sb,
                s2T_sb[64:72].rearrange("p o k -> p (o k)"),
                0.0, -1e30, Alu.is_equal, Alu.mult,
            )
            st["nm_sb"] = nm_sb

            jo_ps = psJ.tile([64, macro], F32, tag="psJ")
            nc.tensor.matmul(jo_ps, jown_sb, s2T_flat, start=True, stop=True)
            concatA = work_p.tile([P, macro], BF16, tag="concatA")
            nc.scalar.activation(
                concatA[0:64, :], jo_ps[0:64], Relu, bias=b0bg_sb[0:64]
            )
            nc.scalar.activation(
                concatA[64:128, :], st["env_ps"], Relu,
                bias=b0bg_sb[64:128], scale=1.0 / 32.0,
            )

            q2_ps = psB.tile([P, macro], F32, tag="psB")
            nc.tensor.matmul(q2_ps, wqk_sb, concatA[0:64, :], start=True, stop=True)
            q2_sb = work_p.tile([P, macro], BF16, tag="q2")
            nc.scalar.copy(q2_sb, q2_ps)
            st["concatA"] = concatA
            st["q2_sb"] = q2_sb

        def emit_B1b2(st):
            """1/den broadcast -> alpha."""
            rdup_sb = sm_p.tile([8, macro], F32R, tag="rdup")
            nc.gpsimd.partition_broadcast(rdup_sb, st["rs_sb"], channels=8)
            alpha_sb = sm_p.tile([8, macro], BF16, tag="alpha")
            nc.gpsimd.tensor_tensor(alpha_sb, st["p8_sb"], rdup_sb, Alu.mult)
            st["alpha_sb"] = alpha_sb

        def emit_F3a(st):
            """i1."""
            i1_sb = blk_p.tile([P, 4, macro], BF16, tag="i1")
            for t in range(2):
                pair_ps = psA2.tile([P, 2, macro], F32, tag="psA2")
                for h in range(2):
                    nc.tensor.matmul(
                        pair_ps[:, h, :], ws1_sb[:, 2 * t + h],
                        st["s2T_flat"], start=True, stop=True,
                    )
                dst = i1_sb[:, 2 * t : 2 * t + 2, :]
                if t == 0:
                    nc.scalar.activation(dst, pair_ps, Relu, bias=bs1_sb)
                else:
                    nc.vector.tensor_scalar(
                        dst, pair_ps, bs1_sb, 0.0, Alu.add, Alu.max
                    )
            st["i1_sb"] = i1_sb

        def emit_B2a(st):
            """Weighted i2 -> h1."""
            i2_sb = st["i2_sb"]; alpha_sb = st["alpha_sb"]

            cmul_sb = blk_p.tile([P, 4, macro], BF16, tag="cmul")
            for t in range(2):
                abc_ps = psA2.tile([P, 2, macro], F32, tag="psA2")
                for h in range(2):
                    nc.tensor.matmul(
                        abc_ps[:, h, :], sel_sb[:, 2 * t + h], alpha_sb,
                        start=True, stop=True,
                    )
                nc.vector.tensor_tensor(
                    cmul_sb[:, 2 * t : 2 * t + 2, :],
                    i2_sb[:, 2 * t : 2 * t + 2, :], abc_ps, Alu.mult,
                )

            h1_ps = psB2.tile([P, macro], F32, tag="psB2")
            nc.tensor.matmul(h1_ps, wc1a_sb, st["concatA"], start=True, stop=False)
            for m in range(4):
                nc.tensor.matmul(
                    h1_ps, wvc_sb, cmul_sb[:, m, :],
                    start=False, stop=(m == 3),
                )
            h1_sb = work_p.tile([P, macro], BF16, tag="h1")
            nc.scalar.activation(h1_sb, h1_ps, Relu, bias=bc1_sb)
            st["h1_sb"] = h1_sb

        def emit_F3b(st):
            """i2, qk."""
            i1_sb = st["i1_sb"]; q2_sb = st["q2_sb"]
            i2_sb = blk_p.tile([P, 4, macro], BF16, tag="i2")
            for t in range(2):
                pair_ps = psA2.tile([P, 2, macro], F32, tag="psA2")
                for h in range(2):
                    nc.tensor.matmul(
                        pair_ps[:, h, :], ws2_sb, i1_sb[:, 2 * t + h, :],
                        start=True, stop=True,
                    )
                dst = i2_sb[:, 2 * t : 2 * t + 2, :]
                if t == 0:
                    nc.scalar.activation(dst, pair_ps, Relu, bias=bs2_sb)
                else:
                    nc.vector.tensor_scalar(
                        dst, pair_ps, bs2_sb, 0.0, Alu.add, Alu.max
                    )

            qk_sb = blk_p.tile([P, 4, macro], BF16, tag="qk")
            nc.gpsimd.tensor_tensor(
                qk_sb[:, 0:2, :], i2_sb[:, 0:2, :],
                q2_sb[:, None, :].to_broadcast((P, 2, macro)),
                Alu.mult,
            )
            nc.gpsimd.tensor_tensor(
                qk_sb[:, 2:4, :], i2_sb[:, 2:4, :],
                q2_sb[:, None, :].to_broadcast((P, 2, macro)),
                Alu.mult,
            )
            st["i2_sb"] = i2_sb
            st["qk_sb"] = qk_sb

        def emit_B3(st):
            """Output transpose into the 128-partition store block."""
            ms = st["row0"] // macro
            oT_ps = psC.tile([P, nb, 2], F32, tag="psC")
            for o in range(nb):
                nc.tensor.transpose(
                    oT_ps[:, o, :], st["o_sb"][:, o * P : (o + 1) * P], ident2_f
                )
            nc.vector.tensor_copy(out128_sb[:, ms], oT_ps)

        def emit_B2b(st):
            """h2."""
            h2_ps = psB2.tile([P, macro], F32, tag="psB2")
            nc.tensor.matmul(h2_ps, wc2_sb, st["h1_sb"], start=True, stop=True)
            h2_sb = work_p.tile([P, macro], BF16, tag="h2")
            nc.scalar.activation(h2_sb, h2_ps, Relu, bias=bc2_sb)
            st["h2_sb"] = h2_sb

        # software pipeline across macros:
        #   iter i: forward phases for macro i, softmax (B1) for i-1,
        #   weighted-sum/head (B2) for i-2, output head for i-3, output
        #   transpose for i-4. Each chain segment gets a full iteration of
        #   slack. DMAs prefetch 2 ahead.
        sts = [None] * (nm + 1)
        sts[0] = emit_F1(0)
        sts[1] = emit_F1(1)

        def stage(ms):
            return sts[ms] if 0 <= ms < nm else None

        for ms in range(nm + 5):
            cur = stage(ms)
            p1, p2, p3, p4 = (
                stage(ms - 1), stage(ms - 2), stage(ms - 3), stage(ms - 4)
            )
            if p1 is not None:
                emit_B1a(p1)
            if cur is not None:
                emit_F1t(cur)
            if ms + 2 < nm:
                sts[ms + 2] = emit_F1(ms + 2)
            if cur is not None:
                emit_F2a(cur)
            if p4 is not None:
                emit_B3(p4)
            if p1 is not None:
                emit_B1b1(p1)
            if cur is not None:
                emit_F2b(cur)
            if p3 is not None:
                emit_OUT(p3)
            if p1 is not None:
                emit_B1b2(p1)
            if cur is not None:
                emit_F3a(cur)
            if p2 is not None:
                emit_B2a(p2)
            if cur is not None:
                emit_F3b(cur)
            if p2 is not None:
                emit_B2b(p2)

        nc.sync.dma_start(
            out_d[:, :], out128_sb.rearrange("p a o k -> p (a o k)")
        )

        for _pool in (psC, psB2, psB, psJ, psA2, blk_p, work_p, sm_p,
                      s2_p, s1T_p, consts):
            _pool.release()

    return nc


def prepare_in_maps(inputs):
    bf = ml_dtypes.bfloat16
    f32 = np.float32
    nm = RPC // MACRO
    nb = MACRO // P

    def a(x, dt=f32):
        return np.ascontiguousarray(np.asarray(x), dtype=dt)

    W0 = a(inputs["W0"]); Wg = a(inputs["Wg"])
    Ws1 = a(inputs["Ws1"]); Ws2 = a(inputs["Ws2"])
    Wq = a(inputs["Wq"]); Wk = a(inputs["Wk"]); Wv = a(inputs["Wv"])
    Wc1 = a(inputs["Wc1"]); Wc2 = a(inputs["Wc2"]); Wc3 = a(inputs["Wc3"])

    f8 = ml_dtypes.float8_e4m3fn
    wqk = Wq @ Wk.T                                   # [64, 64]
    wvc = Wv @ Wc1[128:192, :]                        # [64, 128]

    wpk = np.zeros((P, WPK_COLS), dtype=f32)

    def put(span, arr):
        arr = np.asarray(arr, dtype=f32)
        wpk[0 : arr.shape[0], span[0] : span[0] + arr.shape[1]] = arr

    # wg fp8 hi/lo split, scaled by 32 into e4m3's normal range; layout
    # [p, (hi/lo, c4, t, 64)] with chunk index c = 2*c4 + t
    wg32 = Wg * 32.0
    wg_hi = wg32.astype(f8)
    wg_lo = (wg32 - wg_hi.astype(f32)).astype(f8)
    w8pk = np.stack(
        [
            w.reshape(4, 2, P, 64).transpose(2, 0, 1, 3).reshape(P, 512)
            for w in (wg_hi, wg_lo)
        ],
        axis=1,
    ).reshape(P, 1024)
    # ws1_blk[7n+j, m, 64r+d] = Ws1[j, d] if n == 2m+r (rows 56:62 zero)
    ws1_blk = np.zeros((62, 4, 128), dtype=f32)
    sel = np.zeros((8, 4, 128), dtype=f32)
    for n in range(8):
        m, r = n // 2, n % 2
        ws1_blk[7 * n : 7 * n + 7, m, 64 * r : 64 * r + 64] = Ws1
        sel[n, m, 64 * r : 64 * r + 64] = 1.0
    put(WS1_S, ws1_blk.reshape(62, 512))
    jown = np.zeros((62, 64), dtype=f32)
    jown[56:62, :] = W0
    put(JOWN_S, jown)
    ws2_blk = np.zeros((128, 128), dtype=f32)
    ws2_blk[0:64, 0:64] = Ws2
    ws2_blk[64:128, 64:128] = Ws2
    put(WS2_S, ws2_blk)
    put(WQK_S, np.concatenate([wqk, wqk], axis=1))
    put(WC1A_S, Wc1[0:128, :])
    put(WVC_S, np.concatenate([wvc, wvc], axis=0))
    put(WC2_S, Wc2)
    put(WC3_S, Wc3)
    put(SEL_S, sel.reshape(8, 512))
    scoresel = np.zeros((128, 4, 8), dtype=f32)
    for n in range(8):
        m, r = n // 2, n % 2
        scoresel[64 * r : 64 * r + 64, m, n] = 1.0
    put(SSEL_S, scoresel.reshape(128, 32))
    put(ONES8_S, np.ones((8, 1), dtype=f32))
    put(ID8_S, np.eye(8, dtype=f32))
    put(IDB_S, np.eye(128, dtype=f32))

    def col(x):
        return np.ascontiguousarray(np.asarray(x, dtype=f32).reshape(-1, 1))

    bpk = np.zeros((P, 10), dtype=f32)
    bpk[:, 0:1] = np.concatenate([col(inputs["b0"]), col(inputs["bg"])], axis=0)
    bpk[:, 1:2] = np.concatenate([col(inputs["bs1"])] * 2, axis=0)
    bpk[:, 2:3] = np.concatenate([col(inputs["bs2"])] * 2, axis=0)
    bpk[:, 3:4] = col(inputs["bc1"])
    bpk[:, 4:5] = col(inputs["bc2"])
    bpk[0:2, 5:6] = col(inputs["bc3"])
    bpk[0:2, 6:8] = np.eye(2, dtype=f32)
    bpk[0:64, 8] = 1.0          # own rows: unscaled
    bpk[64:128, 8] = 1.0 / 32.0  # env rows: undo the fp8 weight scaling

    state0 = a(inputs["state0"]); state1 = a(inputs["state1"])
    state2 = a(inputs["state2"])

    # state1 pre-transposed + fp8 hi/lo split: s1t8[h, c, p, b]
    s1t_f = state1.T.reshape(8, P, B_FULL)
    s1_hi = s1t_f.astype(f8)
    s1_lo = (s1t_f - s1_hi.astype(f32)).astype(f8)
    s1t_full = np.ascontiguousarray(np.stack([s1_hi, s1_lo], axis=0))

    # s2aug: per row 128 cols = 56 s2 | 6 s0 | 2 pad | 8 mask slots | pad
    s2aug = np.zeros((B_FULL, 128), dtype=bf)
    s2aug[:, 0:56] = state2.reshape(B_FULL, 56).astype(bf)
    s2aug[:, 56:62] = state0.astype(bf)
    # [core, ms, o, p, 128] -> [core, p, ms, (o 128)]
    s2aug_blk = np.ascontiguousarray(
        s2aug.reshape(N_CORES, nm, nb, P, 128).transpose(0, 3, 1, 2, 4)
        .reshape(N_CORES, P, nm, nb * 128)
    )

    shared = {
        "wpk": a(wpk, bf),
        "w8pk": np.ascontiguousarray(w8pk),
        "bpk": bpk,
    }
    in_maps = []
    for i in range(N_CORES):
        m = dict(shared)
        m["s1t8"] = np.ascontiguousarray(
            s1t_full[:, :, :, i * RPC : (i + 1) * RPC]
        )
        m["s2aug"] = s2aug_blk[i]
        in_maps.append(m)
    return in_maps


def unshard_out(res_core):
    """[128, nm*nb*2] f32 -> [rpc, 2]; out128[p, ms, o, a] = row ms*512+o*128+p."""
    nm = RPC // MACRO
    nb = MACRO // P
    arr = np.asarray(res_core, dtype=np.float32).reshape(P, nm, nb, 2)
    return np.ascontiguousarray(
        arr.transpose(1, 2, 0, 3).reshape(RPC, 2)
    )


_NC_CACHE = {}


def get_nc():
    if "nc" not in _NC_CACHE:
        nc = build()
        nc.finalize()
        _NC_CACHE["nc"] = nc
    return _NC_CACHE["nc"]


def kernel(**inputs):
    nc = get_nc()
    in_maps = prepare_in_maps(inputs)
    trace = bool(int(os.environ.get("K_TRACE", "0")))
    try:
        res = run_bass_kernel_spmd(
            nc, in_maps, core_ids=list(range(N_CORES)), trace=trace
        )
    except ModuleNotFoundError:
        res = run_bass_kernel_spmd(nc, in_maps, core_ids=list(range(N_CORES)))
    if res.exec_time_ns is not None:
        print(f"HW exec time: {res.exec_time_ns} ns")
    parts = [unshard_out(res.results[i]["out"]) for i in range(N_CORES)]
    return np.ascontiguousarray(np.concatenate(parts, axis=0))


# revision 86
# speedup vs baseline: 1.0874x; 1.0377x over previous
"""Trainium2 Bass kernel for nn_ActorNetwork (gnn_message_passing).

Pure data-parallel across 8 NeuronCores: each core processes 8192 of the
65536 batch rows; small weights are replicated.

v6: feature-major, neighbor-pair packing; deep software pipeline tuned so
the in-order PE stream never waits on slower engines:
  - state1 uploaded pre-transposed (bf16 [8, 128, rpc]); prefetched two
    macros ahead.
  - state0 + state2 host-packed into one 64-col bf16 block per row
    (56 s2 | 6 s0 | 2 pad); per-neighbor mask sums via a jsum matmul after
    the PE transpose; mask folded into the score PSUM via an identity
    matmul over the -1e30 rows.
  - outputs PE-transposed into a [128, 128] per-core block -> one 512B/part
    store DMA; the out-head matmul + tanh + transpose are pipelined 2-3
    macros behind compute.
  - all small weights packed into two DMAs issued on the Act queue.
  - elementwise work balanced across Act / DVE / Pool.
"""

import os

import numpy as np
import ml_dtypes

import concourse.bass as bass
import concourse.tile as tile
from concourse import bacc
from concourse import mybir
from concourse.bass_utils import run_bass_kernel_spmd

F32 = mybir.dt.float32
F32R = mybir.dt.float32r
BF16 = mybir.dt.bfloat16
F8 = mybir.dt.float8e4
DR = mybir.MatmulPerfMode.DoubleRow

N_CORES = 8
B_FULL = 65536
RPC = B_FULL // N_CORES        # rows per core = 8192
MACRO = 512                    # batch rows per macro tile
P = 128

Relu = mybir.ActivationFunctionType.Relu
Tanh = mybir.ActivationFunctionType.Tanh
Exp = mybir.ActivationFunctionType.Exp
Alu = mybir.AluOpType
AX = mybir.AxisListType

# column layout of the packed bf16 weight block [128, WPK_COLS]
_off = 0
def _span(n):
    global _off
    s = (_off, _off + n)
    _off += n
    return s
WG_S = _span(512)        # [128, (8c, 64)]
WS1_S = _span(512)       # [62, (4m, 128)]
WS2_S = _span(128)       # [128, 128]
WQK_S = _span(128)       # [64, 128]
WC1A_S = _span(128)      # [128, 128]
WVC_S = _span(128)       # [128, 128]
WC2_S = _span(128)       # [128, 128]
WC3_S = _span(2)         # [128, 2]
SEL_S = _span(512)       # [8, (4m, 128)]
SSEL_S = _span(32)       # [128, (4m, 8)]
ONES8_S = _span(1)       # [8, 1]
ID8_S = _span(8)         # [8, 8]
JOWN_S = _span(64)       # [62, 64] w0 rows 56:62
IDB_S = _span(128)       # [128, 128] bf16 identity
WPK_COLS = _off


def build(rpc=RPC, macro=MACRO):
    nm = rpc // macro          # macro tiles per core = 16
    nb = macro // P            # 128-row blocks per macro tile = 4

    nc = bacc.Bacc()

    s1t_d = nc.declare_dram_parameter("s1t8", [2, 8, P, rpc], F8, isOutput=False)
    s2a_d = nc.declare_dram_parameter("s2aug", [P, nm, nb * 128], BF16, isOutput=False)
    wpk_d = nc.declare_dram_parameter("wpk", [P, WPK_COLS], BF16, isOutput=False)
    w8pk_d = nc.declare_dram_parameter("w8pk", [P, 1024], F8, isOutput=False)
    bpk_d = nc.declare_dram_parameter("bpk", [P, 10], F32, isOutput=False)
    out_d = nc.declare_dram_parameter("out", [P, nm * nb * 2], F32, isOutput=True)

    from concourse import library_config

    with tile.TileContext(nc) as tc:
        nc.gpsimd.load_library(library_config.proxy)
        consts = tc.alloc_tile_pool(name="consts", bufs=1)
        s1T_p = tc.alloc_tile_pool(name="s1T", bufs=3)
        s2_p = tc.alloc_tile_pool(name="s2", bufs=3)
        sm_p = tc.alloc_tile_pool(name="sm", bufs=6)
        work_p = tc.alloc_tile_pool(name="work", bufs=6)
        blk_p = tc.alloc_tile_pool(name="blk", bufs=5)
        psA2 = tc.alloc_tile_pool(name="psA2", bufs=2, space="PSUM")
        psJ = tc.alloc_tile_pool(name="psJ", bufs=1, space="PSUM")
        psB = tc.alloc_tile_pool(name="psB", bufs=1, space="PSUM")
        psB2 = tc.alloc_tile_pool(name="psB2", bufs=1, space="PSUM")
        psC = tc.alloc_tile_pool(name="psC", bufs=1, space="PSUM")

        # ---- packed constants (Act queue; SP starts on state immediately) --
        wpk = consts.tile([P, WPK_COLS], BF16)
        nc.scalar.dma_start(wpk, wpk_d[:, :])
        w8pk = consts.tile([P, 2, 4, 2, 64], F8)
        nc.scalar.dma_start(
            w8pk.rearrange("p h c t m -> p (h c t m)"), w8pk_d[:, :]
        )
        bpk = consts.tile([P, 10], F32)
        nc.scalar.dma_start(bpk, bpk_d[:, :])

        def W(span, rows=P):
            return wpk[0:rows, span[0] : span[1]]

        wg_sb = W(WG_S).rearrange("p (c m) -> p c m", c=8)
        ws1_sb = W(WS1_S, 62).rearrange("p (m k) -> p m k", m=4)
        ws2_sb = W(WS2_S)
        wqk_sb = W(WQK_S, 64)
        wc1a_sb = W(WC1A_S)
        wvc_sb = W(WVC_S)
        wc2_sb = W(WC2_S)
        wc3_sb = W(WC3_S)
        sel_sb = W(SEL_S, 8).rearrange("p (m k) -> p m k", m=4)
        ssel_sb = W(SSEL_S).rearrange("p (m k) -> p m k", m=4)
        ones8_sb = W(ONES8_S, 8)
        ident8_sb = W(ID8_S, 8)
        jown_sb = W(JOWN_S, 62)
        ident_b = W(IDB_S)

        b0bg_sb = bpk[:, 0:1]
        bs1_sb = bpk[:, 1:2]
        bs2_sb = bpk[:, 2:3]
        bc1_sb = bpk[:, 3:4]
        bc2_sb = bpk[:, 4:5]
        bc3_sb = bpk[0:2, 5:6]
        ident2_f = bpk[0:2, 6:8]

        out128_sb = consts.tile([P, nm, nb, 2], F32)

        def emit_F1(ms):
            """State loads for macro ms (prefetched two macros ahead)."""
            row0 = ms * macro
            s2a = s2_p.tile([P, nb, 128], BF16, tag="s2a")
            nc.sync.dma_start(
                s2a.rearrange("p o k -> p (o k)"), s2a_d[:, ms, :]
            )
            s1T = s1T_p.tile([P, 2, 8, macro], F8, tag="s1T")
            nc.sync.dma_start(
                s1T,
                s1t_d[:, :, :, row0 : row0 + macro].rearrange(
                    "h c p b -> p h c b"
                ),
            )
            return dict(row0=row0, s1T=s1T, s2a=s2a)

        def emit_OUT(st):
            """Output head + tanh (for macro m-2)."""
            o_ps = psB.tile([2, macro], F32, tag="psB")
            nc.tensor.matmul(o_ps, wc3_sb, st["h2_sb"], start=True, stop=True)
            o_sb = sm_p.tile([2, macro], F32, tag="osb")
            nc.scalar.activation(o_sb, o_ps, Tanh, bias=bc3_sb)
            st["o_sb"] = o_sb

        def emit_B1a(st):
            """Scores (incl. mask) -> exp."""
            sc_ps = psC.tile([8, macro], F32, tag="psC")
            for m in range(4):
                nc.tensor.matmul(
                    sc_ps, ssel_sb[:, m, :], st["qk_sb"][:, m, :],
                    start=(m == 0), stop=False,
                )
            nc.tensor.matmul(sc_ps, ident8_sb, st["nm_sb"], start=False, stop=True)
            p8_sb = sm_p.tile([8, macro], BF16, tag="p8")
            nc.scalar.activation(p8_sb, sc_ps, Exp, scale=0.125)
            st["p8_sb"] = p8_sb

        def emit_F1t(st):
            """mask sums (batch-major) + xbar DMA transposes straight to SBUF."""
            s2a = st["s2a"]
            with nc.allow_low_precision(reason="mask sums only compared to 0"):
                nc.vector.tensor_reduce(
                    s2a[:, :, 64:72],
                    s2a[:, :, 0:56].rearrange("p o (n j) -> p o n j", j=7),
                    AX.X,
                    Alu.add,
                )
            s2T_sb = sm_p.tile([P, nb, P], BF16, tag="s2Tsb")
            for o in range(nb):
                nc.sync.dma_start_transpose(s2T_sb[:, o, :], s2a[:, o, :])
            st["s2T_sb"] = s2T_sb

        def emit_F2a(st):
            """env matmuls: fp8 DoubleRow, hi@Wa + hi@Wb + lo@Wa."""
            env_ps = psB.tile([64, macro], F32, tag="psB")
            s1T = st["s1T"]
            first = True
            for h, w in ((0, 0), (0, 1)):
                for c4 in range(4):
                    nc.tensor.matmul(
                        env_ps,
                        w8pk[:, w, c4],
                        s1T[:, h, 2 * c4 : 2 * c4 + 2, :],
                        start=first, stop=(h, w, c4) == (0, 1, 3),
                        perf_mode=DR,
                    )
                    first = False
            st["env_ps"] = env_ps

        def emit_B1b1(st):
            """Softmax denominator."""
            den_ps = psC.tile([1, macro], F32, tag="psC")
            nc.tensor.matmul(den_ps, ones8_sb, st["p8_sb"], start=True, stop=True)
            rs_sb = sm_p.tile([1, macro], F32R, tag="rs")
            with nc.allow_low_precision(reason="f32r reciprocal, 19-bit ok"):
                nc.vector.reciprocal(rs_sb, den_ps)
            st["rs_sb"] = rs_sb

        def emit_F2b(st):
            """own matmul; mask rows; concatA; q2."""
            s2T_sb = st["s2T_sb"]
            s2T_flat = s2T_sb[0:62].rearrange("p o k -> p (o k)")
            st["s2T_flat"] = s2T_flat

            nm_sb = sm_p.tile([8, macro], BF16, tag="nm")
            nc.vector.tensor_scalar(
                nm_sb,
                s2T_sb[64:72].rearrange("p o k -> p (o k)"),
                0.0, -1e30, Alu.is_equal, Alu.mult,
            )
            st["nm_sb"] = nm_sb

            jo_ps = psJ.tile([64, macro], F32, tag="psJ")
            nc.tensor.matmul(jo_ps, jown_sb, s2T_flat, start=True, stop=True)
            concatA = work_p.tile([P, macro], BF16, tag="concatA")
            nc.scalar.activation(
                concatA[0:64, :], jo_ps[0:64], Relu, bias=b0bg_sb[0:64]
            )
            nc.scalar.activation(
                concatA[64:128, :], st["env_ps"], Relu,
                bias=b0bg_sb[64:128], scale=1.0 / 32.0,
            )

            q2_ps = psB.tile([P, macro], F32, tag="psB")
            nc.tensor.matmul(q2_ps, wqk_sb, concatA[0:64, :], start=True, stop=True)
            q2_sb = work_p.tile([P, macro], BF16, tag="q2")
            nc.scalar.copy(q2_sb, q2_ps)
            st["concatA"] = concatA
            st["q2_sb"] = q2_sb

        def emit_B1b2(st):
            """1/den broadcast -> alpha."""
            rdup_sb = sm_p.tile([8, macro], F32R, tag="rdup")
            nc.gpsimd.partition_broadcast(rdup_sb, st["rs_sb"], channels=8)
            alpha_sb = sm_p.tile([8, macro], BF16, tag="alpha")
            nc.gpsimd.tensor_tensor(alpha_sb, st["p8_sb"], rdup_sb, Alu.mult)
            st["alpha_sb"] = alpha_sb

        def emit_F3a(st):
            """i1."""
            i1_sb = blk_p.tile([P, 4, macro], BF16, tag="i1")
            for t in range(2):
                pair_ps = psA2.tile([P, 2, macro], F32, tag="psA2")
                for h in range(2):
                    nc.tensor.matmul(
                        pair_ps[:, h, :], ws1_sb[:, 2 * t + h],
                        st["s2T_flat"], start=True, stop=True,
                    )
                dst = i1_sb[:, 2 * t : 2 * t + 2, :]
                if t == 0:
                    nc.scalar.activation(dst, pair_ps, Relu, bias=bs1_sb)
                else:
                    nc.vector.tensor_scalar(
                        dst, pair_ps, bs1_sb, 0.0, Alu.add, Alu.max
                    )
            st["i1_sb"] = i1_sb

        def emit_B2a(st):
            """Weighted i2 -> h1."""
            i2_sb = st["i2_sb"]; alpha_sb = st["alpha_sb"]

            cmul_sb = blk_p.tile([P, 4, macro], BF16, tag="cmul")
            for t in range(2):
                abc_ps = psA2.tile([P, 2, macro], F32, tag="psA2")
                for h in range(2):
                    nc.tensor.matmul(
                        abc_ps[:, h, :], sel_sb[:, 2 * t + h], alpha_sb,
                        start=True, stop=True,
                    )
                nc.vector.tensor_tensor(
                    cmul_sb[:, 2 * t : 2 * t + 2, :],
                    i2_sb[:, 2 * t : 2 * t + 2, :], abc_ps, Alu.mult,
                )

            h1_ps = psB2.tile([P, macro], F32, tag="psB2")
            nc.tensor.matmul(h1_ps, wc1a_sb, st["concatA"], start=True, stop=False)
            for m in range(4):
                nc.tensor.matmul(
                    h1_ps, wvc_sb, cmul_sb[:, m, :],
                    start=False, stop=(m == 3),
                )
            h1_sb = work_p.tile([P, macro], BF16, tag="h1")
            nc.scalar.activation(h1_sb, h1_ps, Relu, bias=bc1_sb)
            st["h1_sb"] = h1_sb

        def emit_F3b(st):
            """i2, qk."""
            i1_sb = st["i1_sb"]; q2_sb = st["q2_sb"]
            i2_sb = blk_p.tile([P, 4, macro], BF16, tag="i2")
            for t in range(2):
                pair_ps = psA2.tile([P, 2, macro], F32, tag="psA2")
                for h in range(2):
                    nc.tensor.matmul(
                        pair_ps[:, h, :], ws2_sb, i1_sb[:, 2 * t + h, :],
                        start=True, stop=True,
                    )
                dst = i2_sb[:, 2 * t : 2 * t + 2, :]
                if t == 0:
                    nc.scalar.activation(dst, pair_ps, Relu, bias=bs2_sb)
                else:
                    nc.vector.tensor_scalar(
                        dst, pair_ps, bs2_sb, 0.0, Alu.add, Alu.max
                    )

            qk_sb = blk_p.tile([P, 4, macro], BF16, tag="qk")
            nc.gpsimd.tensor_tensor(
                qk_sb[:, 0:2, :], i2_sb[:, 0:2, :],
                q2_sb[:, None, :].to_broadcast((P, 2, macro)),
                Alu.mult,
            )
            nc.gpsimd.tensor_tensor(
                qk_sb[:, 2:4, :], i2_sb[:, 2:4, :],
                q2_sb[:, None, :].to_broadcast((P, 2, macro)),
                Alu.mult,
            )
            st["i2_sb"] = i2_sb
            st["qk_sb"] = qk_sb

        def emit_B3(st):
            """Output transpose into the 128-partition store block."""
            ms = st["row0"] // macro
            oT_ps = psC.tile([P, nb, 2], F32, tag="psC")
            for o in range(nb):
                nc.tensor.transpose(
                    oT_ps[:, o, :], st["o_sb"][:, o * P : (o + 1) * P], ident2_f
                )
            nc.vector.tensor_copy(out128_sb[:, ms], oT_ps)

        def emit_B2b(st):
            """h2."""
            h2_ps = psB2.tile([P, macro], F32, tag="psB2")
            nc.tensor.matmul(h2_ps, wc2_sb, st["h1_sb"], start=True, stop=True)
            h2_sb = work_p.tile([P, macro], BF16, tag="h2")
            nc.scalar.activation(h2_sb, h2_ps, Relu, bias=bc2_sb)
            st["h2_sb"] = h2_sb

        # software pipeline across macros:
        #   iter i: forward phases for macro i, softmax (B1) for i-1,
        #   weighted-sum/head (B2) for i-2, output head for i-3, output
        #   transpose for i-4. Each chain segment gets a full iteration of
        #   slack. DMAs prefetch 2 ahead.
        sts = [None] * (nm + 1)
        sts[0] = emit_F1(0)
        sts[1] = emit_F1(1)

        def stage(ms):
            return sts[ms] if 0 <= ms < nm else None

        for ms in range(nm + 5):
            cur = stage(ms)
            p1, p2, p3, p4 = (
                stage(ms - 1), stage(ms - 2), stage(ms - 3), stage(ms - 4)
            )
            if p1 is not None:
                emit_B1a(p1)
            if cur is not None:
                emit_F1t(cur)
            if ms + 2 < nm:
                sts[ms + 2] = emit_F1(ms + 2)
            if cur is not None:
                emit_F2a(cur)
            if p4 is not None:
                emit_B3(p4)
            if p1 is not None:
                emit_B1b1(p1)
            if cur is not None:
                emit_F2b(cur)
            if p3 is not None:
                emit_OUT(p3)
            if p1 is not None:
                emit_B1b2(p1)
            if cur is not None:
                emit_F3a(cur)
            if p2 is not None:
                emit_B2a(p2)
            if cur is not None:
                emit_F3b(cur)
            if p2 is not None:
                emit_B2b(p2)

        nc.sync.dma_start(
            out_d[:, :], out128_sb.rearrange("p a o k -> p (a o k)")
        )

        for _pool in (psC, psB2, psB, psJ, psA2, blk_p, work_p, sm_p,
                      s2_p, s1T_p, consts):
            _pool.release()

    return nc


def prepare_in_maps(inputs):
    bf = ml_dtypes.bfloat16
    f32 = np.float32
    nm = RPC // MACRO
    nb = MACRO // P

    def a(x, dt=f32):
        return np.ascontiguousarray(np.asarray(x), dtype=dt)

    W0 = a(inputs["W0"]); Wg = a(inputs["Wg"])
    Ws1 = a(inputs["Ws1"]); Ws2 = a(inputs["Ws2"])
    Wq = a(inputs["Wq"]); Wk = a(inputs["Wk"]); Wv = a(inputs["Wv"])
    Wc1 = a(inputs["Wc1"]); Wc2 = a(inputs["Wc2"]); Wc3 = a(inputs["Wc3"])

    f8 = ml_dtypes.float8_e4m3fn
    wqk = Wq @ Wk.T                                   # [64, 64]
    wvc = Wv @ Wc1[128:192, :]                        # [64, 128]

    wpk = np.zeros((P, WPK_COLS), dtype=f32)

    def put(span, arr):
        arr = np.asarray(arr, dtype=f32)
        wpk[0 : arr.shape[0], span[0] : span[0] + arr.shape[1]] = arr

    # wg fp8 hi/lo split, scaled by 32 into e4m3's normal range; layout
    # [p, (hi/lo, c4, t, 64)] with chunk index c = 2*c4 + t
    wg32 = Wg * 32.0
    wg_hi = wg32.astype(f8)
    wg_lo = (wg32 - wg_hi.astype(f32)).astype(f8)
    w8pk = np.stack(
        [
            w.reshape(4, 2, P, 64).transpose(2, 0, 1, 3).reshape(P, 512)
            for w in (wg_hi, wg_lo)
        ],
        axis=1,
    ).reshape(P, 1024)
    # ws1_blk[7n+j, m, 64r+d] = Ws1[j, d] if n == 2m+r (rows 56:62 zero)
    ws1_blk = np.zeros((62, 4, 128), dtype=f32)
    sel = np.zeros((8, 4, 128), dtype=f32)
    for n in range(8):
        m, r = n // 2, n % 2
        ws1_blk[7 * n : 7 * n + 7, m, 64 * r : 64 * r + 64] = Ws1
        sel[n, m, 64 * r : 64 * r + 64] = 1.0
    put(WS1_S, ws1_blk.reshape(62, 512))
    jown = np.zeros((62, 64), dtype=f32)
    jown[56:62, :] = W0
    put(JOWN_S, jown)
    ws2_blk = np.zeros((128, 128), dtype=f32)
    ws2_blk[0:64, 0:64] = Ws2
    ws2_blk[64:128, 64:128] = Ws2
    put(WS2_S, ws2_blk)
    put(WQK_S, np.concatenate([wqk, wqk], axis=1))
    put(WC1A_S, Wc1[0:128, :])
    put(WVC_S, np.concatenate([wvc, wvc], axis=0))
    put(WC2_S, Wc2)
    put(WC3_S, Wc3)
    put(SEL_S, sel.reshape(8, 512))
    scoresel = np.zeros((128, 4, 8), dtype=f32)
    for n in range(8):
        m, r = n // 2, n % 2
        scoresel[64 * r : 64 * r + 64, m, n] = 1.0
    put(SSEL_S, scoresel.reshape(128, 32))
    put(ONES8_S, np.ones((8, 1), dtype=f32))
    put(ID8_S, np.eye(8, dtype=f32))
    put(IDB_S, np.eye(128, dtype=f32))

    def col(x):
        return np.ascontiguousarray(np.asarray(x, dtype=f32).reshape(-1, 1))

    bpk = np.zeros((P, 10), dtype=f32)
    bpk[:, 0:1] = np.concatenate([col(inputs["b0"]), col(inputs["bg"])], axis=0)
    bpk[:, 1:2] = np.concatenate([col(inputs["bs1"])] * 2, axis=0)
    bpk[:, 2:3] = np.concatenate([col(inputs["bs2"])] * 2, axis=0)
    bpk[:, 3:4] = col(inputs["bc1"])
    bpk[:, 4:5] = col(inputs["bc2"])
    bpk[0:2, 5:6] = col(inputs["bc3"])
    bpk[0:2, 6:8] = np.eye(2, dtype=f32)
    bpk[0:64, 8] = 1.0          # own rows: unscaled
    bpk[64:128, 8] = 1.0 / 32.0  # env rows: undo the fp8 weight scaling

    state0 = a(inputs["state0"]); state1 = a(inputs["state1"])
    state2 = a(inputs["state2"])

    # state1 pre-transposed + fp8 hi/lo split: s1t8[h, c, p, b]
    s1t_f = state1.T.reshape(8, P, B_FULL)
    s1_hi = s1t_f.astype(f8)
    s1_lo = (s1t_f - s1_hi.astype(f32)).astype(f8)
    s1t_full = np.ascontiguousarray(np.stack([s1_hi, s1_lo], axis=0))

    # s2aug: per row 128 cols = 56 s2 | 6 s0 | 2 pad | 8 mask slots | pad
    s2aug = np.zeros((B_FULL, 128), dtype=bf)
    s2aug[:, 0:56] = state2.reshape(B_FULL, 56).astype(bf)
    s2aug[:, 56:62] = state0.astype(bf)
    # [core, ms, o, p, 128] -> [core, p, ms, (o 128)]
    s2aug_blk = np.ascontiguousarray(
        s2aug.reshape(N_CORES, nm, nb, P, 128).transpose(0, 3, 1, 2, 4)
        .reshape(N_CORES, P, nm, nb * 128)
    )

    shared = {
        "wpk": a(wpk, bf),
        "w8pk": np.ascontiguousarray(w8pk),
        "bpk": bpk,
    }
    in_maps = []
    for i in range(N_CORES):
        m = dict(shared)
        m["s1t8"] = np.ascontiguousarray(
            s1t_full[:, :, :, i * RPC : (i + 1) * RPC]
        )
        m["s2aug"] = s2aug_blk[i]
        in_maps.append(m)
    return in_maps


def unshard_out(res_core):
    """[128, nm*nb*2] f32 -> [rpc, 2]; out128[p, ms, o, a] = row ms*512+o*128+p."""
    nm = RPC // MACRO
    nb = MACRO // P
    arr = np.asarray(res_core, dtype=np.float32).reshape(P, nm, nb, 2)
    return np.ascontiguousarray(
        arr.transpose(1, 2, 0, 3).reshape(RPC, 2)
    )


_NC_CACHE = {}


def get_nc():
    if "nc" not in _NC_CACHE:
        nc = build()
        nc.finalize()
        _NC_CACHE["nc"] = nc
    return _NC_CACHE["nc"]


def kernel(**inputs):
    nc = get_nc()
    in_maps = prepare_in_maps(inputs)
    trace = bool(int(os.environ.get("K_TRACE", "0")))
    try:
        res = run_bass_kernel_spmd(
            nc, in_maps, core_ids=list(range(N_CORES)), trace=trace
        )
    except ModuleNotFoundError:
        res = run_bass_kernel_spmd(nc, in_maps, core_ids=list(range(N_CORES)))
    if res.exec_time_ns is not None:
        print(f"HW exec time: {res.exec_time_ns} ns")
    parts = [unshard_out(res.results[i]["out"]) for i in range(N_CORES)]
    return np.ascontiguousarray(np.concatenate(parts, axis=0))
